# revision 4
# baseline (speedup 1.0000x reference)
"""Trainium2 Bass kernel for the BDH dense-transformer problem.

Sharding: 8 cores = 4 heads x 2 sequence-halves. Each core owns one head and
512 of the 1024 sequence rows ("own" rows live at permuted positions [0,512)).
Core c = (head h=c//2, group g=c%2); group 1 sees all T-indexed data with the
two 512-halves swapped so the program is identical on every core (pure SPMD,
per-core differences are input data only). After the per-head MLP partial, a
single 8-core AllReduce of the canonical [1024,192] ymlp buffer combines heads
and halves; selection masks (0/1 scalars) route permuted<->canonical layouts.

All matmul/elementwise storage is fp16 (PE upconverts to its FP22 internal
format, accumulates fp32 in PSUM); LayerNorm stats and residual math run fp32
inside the engines. Verified in numpy simulation: ~1e-3 rel error vs fp32 ref.
"""

import math

import numpy as np

P = 128
T = 1024
D = 192
NH = 4
N = 3072
NPAIR = 1536
NPC = 12          # 128-row chunks of the 1536 rope pairs
VOCAB = 256
EPS = 1e-5
N_LAYER = 4
NCORES = 8
HALF = 512
TBLK = T // P     # 8 canonical 128-row blocks

_CACHE = {}


def _get_freqs(n, theta=2.0 ** 16):
    t = np.arange(n, dtype=np.float32)
    q = np.floor(t / 2.0) * 2.0
    return (1.0 / theta ** (q / n) / (2.0 * math.pi)).astype(np.float32)


def _ln_np(x):
    m = x.mean(-1, keepdims=True)
    v = x.var(-1, keepdims=True)
    return (x - m) / np.sqrt(v + EPS)


def build_program(repeat=1):
    """Build (and cache) the SPMD Bass program. repeat>1 runs the whole
    network that many times back-to-back (for differential timing)."""
    key = ("nc", repeat)
    if key in _CACHE:
        return _CACHE[key]

    import concourse.mybir as mybir
    import concourse.tile as tile
    from concourse import bacc

    f16 = mybir.dt.float16
    f32 = mybir.dt.float32
    AF = mybir.ActivationFunctionType
    OP = mybir.AluOpType

    nc = bacc.Bacc("TRN2", target_bir_lowering=False, debug=False,
                   num_devices=NCORES)

    # ---- I/O ----
    x0_d = nc.dram_tensor("x0", [T, D], f16, kind="ExternalInput")
    wxe_d = nc.dram_tensor("wxe", [D, NPAIR], f16, kind="ExternalInput")
    wxo_d = nc.dram_tensor("wxo", [D, NPAIR], f16, kind="ExternalInput")
    wye_d = nc.dram_tensor("wye", [D, NPAIR], f16, kind="ExternalInput")
    wyo_d = nc.dram_tensor("wyo", [D, NPAIR], f16, kind="ExternalInput")
    ence_d = nc.dram_tensor("ence", [NPAIR, D], f16, kind="ExternalInput")
    enco_d = nc.dram_tensor("enco", [NPAIR, D], f16, kind="ExternalInput")
    cos_d = nc.dram_tensor("cosT", [NPAIR, T], f16, kind="ExternalInput")
    sin_d = nc.dram_tensor("sinT", [NPAIR, T], f16, kind="ExternalInput")
    m0_d = nc.dram_tensor("m0", [P, P], f16, kind="ExternalInput")
    gsel_d = nc.dram_tensor("gsel", [P, 2], f32, kind="ExternalInput")
    lmh_d = nc.dram_tensor("lmh", [D, VOCAB], f16, kind="ExternalInput")
    logits_d = nc.dram_tensor("logits", [T, VOCAB], f32, kind="ExternalOutput")

    with tile.TileContext(nc) as tc:
        with (
            tc.tile_pool(name="const", bufs=1) as cpool,
            tc.tile_pool(name="state", bufs=1) as spool,
            tc.tile_pool(name="work", bufs=2) as work,
            tc.tile_pool(name="stats", bufs=4) as stp,
            tc.tile_pool(name="psA", bufs=4, space="PSUM") as psA,
            tc.tile_pool(name="psB", bufs=4, space="PSUM") as psB,
            tc.tile_pool(name="dram", bufs=1, space="DRAM") as dpool,
        ):
            # ---- persistent SBUF residents ----
            wxe_a = cpool.tile([P, NPAIR], f16, tag="wxe_a")
            wxe_b = cpool.tile([P, NPAIR], f16, tag="wxe_b")
            wxo_a = cpool.tile([P, NPAIR], f16, tag="wxo_a")
            wxo_b = cpool.tile([P, NPAIR], f16, tag="wxo_b")
            wye_a = cpool.tile([P, NPAIR], f16, tag="wye_a")
            wye_b = cpool.tile([P, NPAIR], f16, tag="wye_b")
            wyo_a = cpool.tile([P, NPAIR], f16, tag="wyo_a")
            wyo_b = cpool.tile([P, NPAIR], f16, tag="wyo_b")
            ence_t = cpool.tile([P, NPC, D], f16, tag="ence")
            enco_t = cpool.tile([P, NPC, D], f16, tag="enco")
            cos_t = cpool.tile([P, NPC, T], f16, tag="cos")
            sin_t = cpool.tile([P, NPC, T], f16, tag="sin")
            m0_t = cpool.tile([P, P], f16, tag="m0")
            gsel_t = cpool.tile([P, 2], f32, tag="gsel")
            eps_t = cpool.tile([P, 1], f32, tag="eps")
            lmh_a = cpool.tile([P, VOCAB], f16, tag="lmh_a")
            lmh_b = cpool.tile([P, VOCAB], f16, tag="lmh_b")

            qrE = spool.tile([P, NPC, T], f16, tag="qrE")
            qrO = spool.tile([P, NPC, T], f16, tag="qrO")
            Eown = spool.tile([P, NPC, HALF], f16, tag="Eown")
            Oown = spool.tile([P, NPC, HALF], f16, tag="Oown")
            x16 = spool.tile([P, TBLK, D], f16, tag="x16")
            xT0 = spool.tile([P, T], f16, tag="xT0")
            xT1 = spool.tile([P, T], f16, tag="xT1")
            scT = spool.tile([P, TBLK, HALF], f16, tag="scT")
            ykvT0 = spool.tile([P, HALF], f16, tag="ykvT0")
            ykvT1 = spool.tile([P, HALF], f16, tag="ykvT1")

            x16_dram = dpool.tile([T, D], f16)
            ykv_dram = dpool.tile([HALF, D], f16)
            bounce_in = dpool.tile([T, D], f32)
            bounce_out = dpool.tile([T, D], f32)

            # ---- load constants ----
            nc.sync.dma_start(wxe_a[:, :], wxe_d[0:P, :])
            nc.sync.dma_start(wxe_b[64:P, :], wxe_d[P:D, :])
            nc.sync.dma_start(wxo_a[:, :], wxo_d[0:P, :])
            nc.sync.dma_start(wxo_b[64:P, :], wxo_d[P:D, :])
            nc.sync.dma_start(wye_a[:, :], wye_d[0:P, :])
            nc.sync.dma_start(wye_b[64:P, :], wye_d[P:D, :])
            nc.sync.dma_start(wyo_a[:, :], wyo_d[0:P, :])
            nc.sync.dma_start(wyo_b[64:P, :], wyo_d[P:D, :])
            nc.sync.dma_start(lmh_a[:, :], lmh_d[0:P, :])
            nc.sync.dma_start(lmh_b[64:P, :], lmh_d[P:D, :])
            nc.sync.dma_start(m0_t[:, :], m0_d[:, :])
            nc.sync.dma_start(gsel_t[:, :], gsel_d[:, :])
            for pc in range(NPC):
                nc.sync.dma_start(ence_t[:, pc, :], ence_d[pc * P:(pc + 1) * P, :])
                nc.sync.dma_start(enco_t[:, pc, :], enco_d[pc * P:(pc + 1) * P, :])
                nc.sync.dma_start(cos_t[:, pc, :], cos_d[pc * P:(pc + 1) * P, :])
                nc.sync.dma_start(sin_t[:, pc, :], sin_d[pc * P:(pc + 1) * P, :])

            nc.vector.memset(eps_t[:, :], EPS)

            # zero regions of masked score strips (stay zero forever)
            for s in range(1, 4):
                nc.vector.memset(scT[:, s, 0:s * P], 0)

            def ln_vecs(src_ap):
                """LayerNorm stats of src [P,F] -> (r, negmr) [P,1] f32."""
                st = stp.tile([P, 6], f32, tag="bnst")
                nc.vector.bn_stats(st[:, :], src_ap)
                mv = stp.tile([P, 2], f32, tag="bnmv")
                nc.vector.bn_aggr(mv[:, :], st[:, :])
                sd = stp.tile([P, 1], f32, tag="sd")
                nc.scalar.activation(sd[:, :], mv[:, 1:2], AF.Sqrt, bias=eps_t[:, :])
                r = stp.tile([P, 1], f32, tag="r")
                nc.vector.reciprocal(r[:, :], sd[:, :])
                nmr = stp.tile([P, 1], f32, tag="nmr")
                nc.vector.tensor_mul(nmr[:, :], mv[:, 0:1], r[:, :])
                nc.vector.tensor_scalar_mul(nmr[:, :], nmr[:, :], -1.0)
                return r, nmr

            def load_x_and_transpose(first):
                if first:
                    for cb in range(TBLK):
                        nc.sync.dma_start(x16[:, cb, :], x0_d[cb * P:(cb + 1) * P, :])
                for cb in range(TBLK):
                    nc.sync.dma_start(x16_dram[cb * P:(cb + 1) * P, :], x16[:, cb, :])
                nc.sync.dma_start_transpose(xT0[:, :], x16_dram[:, 0:P])
                nc.sync.dma_start_transpose(xT1[:, :], x16_dram[:, 64:D])

            def layer():
                # ---- x_sparse + rope ----
                for pc in range(NPC):
                    pcs = slice(pc * P, (pc + 1) * P)
                    psE0 = psA.tile([P, HALF], f32, tag="big")
                    psE1 = psA.tile([P, HALF], f32, tag="big")
                    psO0 = psA.tile([P, HALF], f32, tag="big")
                    psO1 = psA.tile([P, HALF], f32, tag="big")
                    nc.tensor.matmul(psE0[:, :], wxe_a[:, pcs], xT0[:, 0:HALF], start=True, stop=False)
                    nc.tensor.matmul(psE0[:, :], wxe_b[64:P, pcs], xT1[64:P, 0:HALF], start=False, stop=True)
                    nc.tensor.matmul(psE1[:, :], wxe_a[:, pcs], xT0[:, HALF:T], start=True, stop=False)
                    nc.tensor.matmul(psE1[:, :], wxe_b[64:P, pcs], xT1[64:P, HALF:T], start=False, stop=True)
                    nc.tensor.matmul(psO0[:, :], wxo_a[:, pcs], xT0[:, 0:HALF], start=True, stop=False)
                    nc.tensor.matmul(psO0[:, :], wxo_b[64:P, pcs], xT1[64:P, 0:HALF], start=False, stop=True)
                    nc.tensor.matmul(psO1[:, :], wxo_a[:, pcs], xT0[:, HALF:T], start=True, stop=False)
                    nc.tensor.matmul(psO1[:, :], wxo_b[64:P, pcs], xT1[64:P, HALF:T], start=False, stop=True)
                    nc.scalar.activation(Eown[:, pc, :], psE0[:, :], AF.Relu)
                    nc.scalar.activation(Oown[:, pc, :], psO0[:, :], AF.Relu)
                    E1 = work.tile([P, HALF], f16, tag="E1")
                    O1 = work.tile([P, HALF], f16, tag="O1")
                    nc.scalar.activation(E1[:, :], psE1[:, :], AF.Relu)
                    nc.scalar.activation(O1[:, :], psO1[:, :], AF.Relu)
                    # qrE = E*c - O*s   (DVE)
                    for half, (Ea, Oa) in ((0, (Eown[:, pc, :], Oown[:, pc, :])),
                                           (1, (E1[:, :], O1[:, :]))):
                        tsl = slice(half * HALF, (half + 1) * HALF)
                        t1 = work.tile([P, HALF], f16, tag="rtE1")
                        t2 = work.tile([P, HALF], f16, tag="rtE2")
                        nc.vector.tensor_mul(t1[:, :], Ea, cos_t[:, pc, tsl])
                        nc.vector.tensor_mul(t2[:, :], Oa, sin_t[:, pc, tsl])
                        nc.vector.tensor_sub(qrE[:, pc, tsl], t1[:, :], t2[:, :])
                        # qrO = O*c + E*s   (GPSIMD, offloaded)
                        t3 = work.tile([P, HALF], f16, tag="rtO1")
                        t4 = work.tile([P, HALF], f16, tag="rtO2")
                        nc.gpsimd.tensor_mul(t3[:, :], Oa, cos_t[:, pc, tsl])
                        nc.gpsimd.tensor_mul(t4[:, :], Ea, sin_t[:, pc, tsl])
                        nc.gpsimd.tensor_add(qrO[:, pc, tsl], t3[:, :], t4[:, :])

                # ---- scores (s-chunk strips over own 512 cols) ----
                for s in range(TBLK):
                    ssl = slice(s * P, (s + 1) * P)
                    psS = psA.tile([P, HALF], f32, tag="big")
                    for pc in range(NPC):
                        nc.tensor.matmul(psS[:, :], qrE[:, pc, ssl], qrE[:, pc, 0:HALF],
                                         start=(pc == 0), stop=False)
                        nc.tensor.matmul(psS[:, :], qrO[:, pc, ssl], qrO[:, pc, 0:HALF],
                                         start=False, stop=(pc == NPC - 1))
                    if s < 4:
                        # [0:s*128) stays zero; diag block masked; right copied
                        nc.vector.tensor_tensor(scT[:, s, s * P:(s + 1) * P],
                                                psS[:, s * P:(s + 1) * P],
                                                m0_t[:, :], OP.mult)
                        if s < 3:
                            nc.scalar.copy(scT[:, s, (s + 1) * P:HALF],
                                           psS[:, (s + 1) * P:HALF])
                    else:
                        # full strip, scaled by [own-is-half1] (0 or 1)
                        nc.scalar.mul(scT[:, s, :], psS[:, :], gsel_t[:, 1:2])

                # ---- ykv + LN + transpose ----
                for tb in range(4):
                    tbs = slice(tb * P, (tb + 1) * P)
                    psY = psA.tile([P, HALF], f32, tag="big")
                    for s in range(TBLK):
                        nc.tensor.matmul(psY[:, 0:D], scT[:, s, tbs], x16[:, s, :],
                                         start=(s == 0), stop=(s == TBLK - 1))
                    r, nmr = ln_vecs(psY[:, 0:D])
                    ykvn = work.tile([P, D], f16, tag="ykvn")
                    nc.scalar.activation(ykvn[:, :], psY[:, 0:D], AF.Identity,
                                         bias=nmr[:, :], scale=r[:, :])
                    nc.sync.dma_start(ykv_dram[tbs, :], ykvn[:, :])
                nc.sync.dma_start_transpose(ykvT0[:, :], ykv_dram[:, 0:P])
                nc.sync.dma_start_transpose(ykvT1[:, :], ykv_dram[:, 64:D])

                # ---- y_sparse, xy, mlp partial ----
                psM = []
                for _mi in range(4):
                    psM_t = psB.tile([P, D], f32, tag="mlp", name=f"psM{_mi}")
                    psM.append(psM_t)
                for side in range(2):
                    wa, wb = (wye_a, wye_b) if side == 0 else (wyo_a, wyo_b)
                    own = Eown if side == 0 else Oown
                    enc_t = ence_t if side == 0 else enco_t
                    for pc in range(NPC):
                        pcs = slice(pc * P, (pc + 1) * P)
                        psYS = psA.tile([P, HALF], f32, tag="big")
                        nc.tensor.matmul(psYS[:, :], wa[:, pcs], ykvT0[:, :], start=True, stop=False)
                        nc.tensor.matmul(psYS[:, :], wb[64:P, pcs], ykvT1[64:P, :], start=False, stop=True)
                        ys = work.tile([P, HALF], f16, tag="ys")
                        nc.scalar.activation(ys[:, :], psYS[:, :], AF.Relu)
                        xy = work.tile([P, HALF], f16, tag="xy")
                        nc.vector.tensor_mul(xy[:, :], own[:, pc, :], ys[:, :])
                        last = (side == 1 and pc == NPC - 1)
                        for tb in range(4):
                            nc.tensor.matmul(psM[tb][:, :], xy[:, tb * P:(tb + 1) * P],
                                             enc_t[:, pc, :],
                                             start=(side == 0 and pc == 0), stop=last)

                # ---- masked scatter to canonical bounce + AllReduce ----
                for tb in range(4):
                    bA = work.tile([P, D], f32, tag="bA")
                    bB = work.tile([P, D], f32, tag="bB")
                    nc.scalar.mul(bA[:, :], psM[tb][:, :], gsel_t[:, 0:1])
                    nc.scalar.mul(bB[:, :], psM[tb][:, :], gsel_t[:, 1:2])
                    nc.sync.dma_start(bounce_in[tb * P:(tb + 1) * P, :], bA[:, :])
                    nc.sync.dma_start(bounce_in[HALF + tb * P:HALF + (tb + 1) * P, :], bB[:, :])
                nc.gpsimd.collective_compute(
                    "AllReduce", OP.add,
                    replica_groups=[list(range(NCORES))],
                    ins=[bounce_in.opt()],
                    outs=[bounce_out.opt()],
                )

                # ---- readback, ln(ymlp), residual, ln, new x16 ----
                for j in range(4):
                    Hj = work.tile([P, D], f32, tag="H0")
                    Hk = work.tile([P, D], f32, tag="H1")
                    nc.sync.dma_start(Hj[:, :], bounce_out[j * P:(j + 1) * P, :])
                    nc.sync.dma_start(Hk[:, :], bounce_out[HALF + j * P:HALF + (j + 1) * P, :])
                    r0, nm0 = ln_vecs(Hj[:, :])
                    r1, nm1 = ln_vecs(Hk[:, :])
                    scaled = []
                    for (rr, nn) in ((r0, nm0), (r1, nm1)):
                        for col in range(2):
                            rs = stp.tile([P, 1], f32, tag="rs")
                            ns = stp.tile([P, 1], f32, tag="ns")
                            nc.vector.tensor_mul(rs[:, :], rr[:, :], gsel_t[:, col:col + 1])
                            nc.vector.tensor_mul(ns[:, :], nn[:, :], gsel_t[:, col:col + 1])
                            scaled.append((rs, ns))
                    # scaled[0]=H0*s0, [1]=H0*s1, [2]=H1*s0, [3]=H1*s1
                    for slot, (ia, ib) in ((j, (0, 3)), (j + 4, (1, 2))):
                        t1a = work.tile([P, D], f32, tag="t1a")
                        t1b = work.tile([P, D], f32, tag="t1b")
                        ra, na = scaled[ia]
                        rb, nb = scaled[ib]
                        nc.scalar.activation(t1a[:, :], Hj[:, :], AF.Identity,
                                             bias=na[:, :], scale=ra[:, :])
                        nc.scalar.activation(t1b[:, :], Hk[:, :], AF.Identity,
                                             bias=nb[:, :], scale=rb[:, :])
                        xm = work.tile([P, D], f32, tag="xm")
                        nc.vector.tensor_add(xm[:, :], t1a[:, :], t1b[:, :])
                        nc.vector.tensor_add(xm[:, :], xm[:, :], x16[:, slot, :])
                        r2, nm2 = ln_vecs(xm[:, :])
                        nc.scalar.activation(x16[:, slot, :], xm[:, :], AF.Identity,
                                             bias=nm2[:, :], scale=r2[:, :])

            for rep in range(repeat):
                load_x_and_transpose(first=True)
                for li in range(N_LAYER):
                    layer()
                    load_x_and_transpose(first=False)

            # ---- lm head ----
            for tb in range(TBLK):
                tbs = slice(tb * P, (tb + 1) * P)
                psL = psA.tile([P, HALF], f32, tag="big")
                nc.tensor.matmul(psL[:, 0:VOCAB], xT0[:, tbs], lmh_a[:, :], start=True, stop=False)
                nc.tensor.matmul(psL[:, 0:VOCAB], xT1[64:P, tbs], lmh_b[64:P, :], start=False, stop=True)
                outL = work.tile([P, VOCAB], f32, tag="outL")
                nc.scalar.copy(outL[:, :], psL[:, 0:VOCAB])
                nc.sync.dma_start(logits_d[tbs, :], outL[:, :])

    nc.compile()
    _CACHE[key] = nc
    return nc


def make_inputs(idx, decoder_x, decoder_y, encoder, embed, pos_emb, lm_head):
    """Host-side prep: per-core input dicts (core c = head c//2, group c%2)."""
    idx = np.asarray(idx)
    decoder_x = np.asarray(decoder_x, dtype=np.float32)
    decoder_y = np.asarray(decoder_y, dtype=np.float32)
    encoder = np.asarray(encoder, dtype=np.float32).reshape(NH, N, D)
    embed = np.asarray(embed, dtype=np.float32)
    pos_emb = np.asarray(pos_emb, dtype=np.float32)
    lm_head = np.asarray(lm_head, dtype=np.float32)

    x0 = _ln_np(embed[idx[0]] + pos_emb[:T]).astype(np.float16)

    freqs = _get_freqs(N)
    fpair = freqs[0::2]
    tt = np.arange(T, dtype=np.float32)
    m0 = np.triu(np.ones((P, P), np.float32), k=1).astype(np.float16)
    lmh = lm_head.astype(np.float16)

    in_maps = []
    for c in range(NCORES):
        h, g = c // 2, c % 2
        tperm = tt if g == 0 else np.concatenate([tt[HALF:], tt[:HALF]])
        ph = ((fpair[:, None] * tperm[None, :]).astype(np.float32) % 1.0) \
            * np.float32(2.0 * math.pi)
        gsel = np.zeros((P, 2), np.float32)
        gsel[:, 0] = 1.0 if g == 0 else 0.0
        gsel[:, 1] = 1.0 - gsel[:, 0]
        x0c = x0 if g == 0 else np.concatenate([x0[HALF:], x0[:HALF]])
        in_maps.append({
            "x0": np.ascontiguousarray(x0c),
            "wxe": np.ascontiguousarray(decoder_x[h][:, 0::2]).astype(np.float16),
            "wxo": np.ascontiguousarray(decoder_x[h][:, 1::2]).astype(np.float16),
            "wye": np.ascontiguousarray(decoder_y[h][:, 0::2]).astype(np.float16),
            "wyo": np.ascontiguousarray(decoder_y[h][:, 1::2]).astype(np.float16),
            "ence": np.ascontiguousarray(encoder[h][0::2]).astype(np.float16),
            "enco": np.ascontiguousarray(encoder[h][1::2]).astype(np.float16),
            "cosT": np.cos(ph.astype(np.float64)).astype(np.float16),
            "sinT": np.sin(ph.astype(np.float64)).astype(np.float16),
            "m0": m0,
            "gsel": gsel,
            "lmh": lmh,
        })
    return in_maps


def kernel(idx, decoder_x, decoder_y, encoder, embed, pos_emb, lm_head):
    from concourse.bass_utils import run_bass_kernel_spmd

    nc = build_program()
    in_maps = make_inputs(idx, decoder_x, decoder_y, encoder, embed, pos_emb,
                          lm_head)
    res = run_bass_kernel_spmd(nc, in_maps, list(range(NCORES)))
    logits = res.results[0]["logits"]  # core 0 uses the identity permutation
    return logits.reshape(1, T, VOCAB).astype(np.float32)


# revision 6
# speedup vs baseline: 1.2479x; 1.2479x over previous
"""Trainium2 Bass kernel for the BDH dense-transformer problem.

Sharding: 8 cores = 4 heads x 2 sequence-halves. Each core owns one head and
512 of the 1024 sequence rows ("own" rows live at permuted positions [0,512)).
Core c = (head h=c//2, group g=c%2); group 1 sees all T-indexed data with the
two 512-halves swapped so the program is identical on every core (pure SPMD,
per-core differences are input data only). After the per-head MLP partial, a
single 8-core AllReduce of the canonical [1024,192] ymlp buffer combines heads
and halves; selection masks (0/1 scalars) route permuted<->canonical layouts.

All matmul/elementwise storage is fp16 (PE upconverts to its FP22 internal
format, accumulates fp32 in PSUM); LayerNorm stats and residual math run fp32
inside the engines. Verified in numpy simulation: ~1e-3 rel error vs fp32 ref.
"""

import math

import numpy as np

P = 128
T = 1024
D = 192
NH = 4
N = 3072
NPAIR = 1536
NPC = 12          # 128-row chunks of the 1536 rope pairs
VOCAB = 256
EPS = 1e-5
N_LAYER = 4
NCORES = 8
HALF = 512
TBLK = T // P     # 8 canonical 128-row blocks

_CACHE = {}


def _get_freqs(n, theta=2.0 ** 16):
    t = np.arange(n, dtype=np.float32)
    q = np.floor(t / 2.0) * 2.0
    return (1.0 / theta ** (q / n) / (2.0 * math.pi)).astype(np.float32)


def _ln_np(x):
    m = x.mean(-1, keepdims=True)
    v = x.var(-1, keepdims=True)
    return (x - m) / np.sqrt(v + EPS)


def build_program(repeat=1):
    """Build (and cache) the SPMD Bass program. repeat>1 runs the whole
    network that many times back-to-back (for differential timing)."""
    key = ("nc", repeat)
    if key in _CACHE:
        return _CACHE[key]

    import concourse.mybir as mybir
    import concourse.tile as tile
    from concourse import bacc

    f16 = mybir.dt.float16
    f32 = mybir.dt.float32
    AF = mybir.ActivationFunctionType
    OP = mybir.AluOpType

    nc = bacc.Bacc("TRN2", target_bir_lowering=False, debug=False,
                   num_devices=NCORES)

    # ---- I/O ----
    x0_d = nc.dram_tensor("x0", [T, D], f16, kind="ExternalInput")
    wxe_d = nc.dram_tensor("wxe", [D, NPAIR], f16, kind="ExternalInput")
    wxo_d = nc.dram_tensor("wxo", [D, NPAIR], f16, kind="ExternalInput")
    wye_d = nc.dram_tensor("wye", [D, NPAIR], f16, kind="ExternalInput")
    wyo_d = nc.dram_tensor("wyo", [D, NPAIR], f16, kind="ExternalInput")
    ence_d = nc.dram_tensor("ence", [NPAIR, D], f16, kind="ExternalInput")
    enco_d = nc.dram_tensor("enco", [NPAIR, D], f16, kind="ExternalInput")
    cos_d = nc.dram_tensor("cosT", [NPAIR, T], f16, kind="ExternalInput")
    sin_d = nc.dram_tensor("sinT", [NPAIR, T], f16, kind="ExternalInput")
    m0_d = nc.dram_tensor("m0", [P, P], f16, kind="ExternalInput")
    gsel_d = nc.dram_tensor("gsel", [P, 2], f32, kind="ExternalInput")
    lmh_d = nc.dram_tensor("lmh", [D, VOCAB], f16, kind="ExternalInput")
    logits_d = nc.dram_tensor("logits", [T, VOCAB], f32, kind="ExternalOutput")

    with tile.TileContext(nc) as tc:
        with (
            tc.tile_pool(name="const", bufs=1) as cpool,
            tc.tile_pool(name="state", bufs=1) as spool,
            tc.tile_pool(name="work", bufs=2) as work,
            tc.tile_pool(name="stats", bufs=4) as stp,
            tc.tile_pool(name="psum", bufs=1, space="PSUM") as psp,
            tc.tile_pool(name="dram", bufs=1, space="DRAM") as dpool,
        ):
            # ---- persistent SBUF residents ----
            wxe_a = cpool.tile([P, NPAIR], f16, tag="wxe_a")
            wxe_b = cpool.tile([P, NPAIR], f16, tag="wxe_b")
            wxo_a = cpool.tile([P, NPAIR], f16, tag="wxo_a")
            wxo_b = cpool.tile([P, NPAIR], f16, tag="wxo_b")
            wye_a = cpool.tile([P, NPAIR], f16, tag="wye_a")
            wye_b = cpool.tile([P, NPAIR], f16, tag="wye_b")
            wyo_a = cpool.tile([P, NPAIR], f16, tag="wyo_a")
            wyo_b = cpool.tile([P, NPAIR], f16, tag="wyo_b")
            ence_t = cpool.tile([P, NPC, D], f16, tag="ence")
            enco_t = cpool.tile([P, NPC, D], f16, tag="enco")
            cos_t = cpool.tile([P, NPC, T], f16, tag="cos")
            sin_t = cpool.tile([P, NPC, T], f16, tag="sin")
            m0_t = cpool.tile([P, P], f16, tag="m0")
            gsel_t = cpool.tile([P, 2], f32, tag="gsel")
            eps_t = cpool.tile([P, 1], f32, tag="eps")
            lmh_a = cpool.tile([P, VOCAB], f16, tag="lmh_a")
            lmh_b = cpool.tile([P, VOCAB], f16, tag="lmh_b")

            qrE = spool.tile([P, NPC, T], f16, tag="qrE")
            qrO = spool.tile([P, NPC, T], f16, tag="qrO")
            Eown = spool.tile([P, NPC, HALF], f16, tag="Eown")
            Oown = spool.tile([P, NPC, HALF], f16, tag="Oown")
            x16 = spool.tile([P, TBLK, D], f16, tag="x16")
            xT0 = spool.tile([P, T], f16, tag="xT0")
            xT1 = spool.tile([P, T], f16, tag="xT1")
            scT = spool.tile([P, TBLK, HALF], f16, tag="scT")
            ykvT0 = spool.tile([P, HALF], f16, tag="ykvT0")
            ykvT1 = spool.tile([P, HALF], f16, tag="ykvT1")

            x16_dram = dpool.tile([T, D], f16)
            ykv_dram = dpool.tile([HALF, D], f16)
            bounce_in = dpool.tile([T, D], f16)
            bounce_out = dpool.tile([T, D], f16)

            # ---- load constants ----
            nc.sync.dma_start(wxe_a[:, :], wxe_d[0:P, :])
            nc.sync.dma_start(wxe_b[64:P, :], wxe_d[P:D, :])
            nc.sync.dma_start(wxo_a[:, :], wxo_d[0:P, :])
            nc.sync.dma_start(wxo_b[64:P, :], wxo_d[P:D, :])
            nc.sync.dma_start(wye_a[:, :], wye_d[0:P, :])
            nc.sync.dma_start(wye_b[64:P, :], wye_d[P:D, :])
            nc.sync.dma_start(wyo_a[:, :], wyo_d[0:P, :])
            nc.sync.dma_start(wyo_b[64:P, :], wyo_d[P:D, :])
            nc.sync.dma_start(lmh_a[:, :], lmh_d[0:P, :])
            nc.sync.dma_start(lmh_b[64:P, :], lmh_d[P:D, :])
            nc.sync.dma_start(m0_t[:, :], m0_d[:, :])
            nc.sync.dma_start(gsel_t[:, :], gsel_d[:, :])
            for pc in range(NPC):
                nc.sync.dma_start(ence_t[:, pc, :], ence_d[pc * P:(pc + 1) * P, :])
                nc.sync.dma_start(enco_t[:, pc, :], enco_d[pc * P:(pc + 1) * P, :])
                nc.sync.dma_start(cos_t[:, pc, :], cos_d[pc * P:(pc + 1) * P, :])
                nc.sync.dma_start(sin_t[:, pc, :], sin_d[pc * P:(pc + 1) * P, :])

            nc.vector.memset(eps_t[:, :], EPS)

            # zero regions of masked score strips (stay zero forever)
            for s in range(1, 4):
                nc.vector.memset(scT[:, s, 0:s * P], 0)

            def ln_vecs(src_ap):
                """LayerNorm stats of src [P,F] -> (r, negmr) [P,1] f32."""
                st = stp.tile([P, 6], f32, tag="bnst")
                nc.vector.bn_stats(st[:, :], src_ap)
                mv = stp.tile([P, 2], f32, tag="bnmv")
                nc.vector.bn_aggr(mv[:, :], st[:, :])
                sd = stp.tile([P, 1], f32, tag="sd")
                nc.scalar.activation(sd[:, :], mv[:, 1:2], AF.Sqrt, bias=eps_t[:, :])
                r = stp.tile([P, 1], f32, tag="r")
                nc.vector.reciprocal(r[:, :], sd[:, :])
                nmr = stp.tile([P, 1], f32, tag="nmr")
                nc.vector.tensor_mul(nmr[:, :], mv[:, 0:1], r[:, :])
                nc.vector.tensor_scalar_mul(nmr[:, :], nmr[:, :], -1.0)
                return r, nmr

            def load_x_and_transpose(first):
                if first:
                    for cb in range(TBLK):
                        nc.sync.dma_start(x16[:, cb, :], x0_d[cb * P:(cb + 1) * P, :])
                for cb in range(TBLK):
                    nc.sync.dma_start(x16_dram[cb * P:(cb + 1) * P, :], x16[:, cb, :])
                nc.sync.dma_start_transpose(xT0[:, :], x16_dram[:, 0:P])
                nc.sync.dma_start_transpose(xT1[:, :], x16_dram[:, 64:D])

            def layer():
                # ---- fused x_sparse / rope / scores ----
                # Wave A: own-half (t' in [0,512)) feeds score strips 0-3.
                # Wave B: other-half feeds strips 4-7. Score-strip matmuls
                # trail the rope by LAG chunks so the PE never waits on DVE.
                LAG = 2
                scps = {}

                def sc_mms(s_lo, ps_list, pc):
                    for si, psS in enumerate(ps_list):
                        s = s_lo + si
                        ssl = slice(s * P, (s + 1) * P)
                        nc.tensor.matmul(psS[:, :], qrE[:, pc, ssl], qrE[:, pc, 0:HALF],
                                         start=(pc == 0), stop=False)
                        nc.tensor.matmul(psS[:, :], qrO[:, pc, ssl], qrO[:, pc, 0:HALF],
                                         start=False, stop=(pc == NPC - 1))

                def sc_drain(s):
                    psS = scps[s]
                    if s < 4:
                        nc.vector.tensor_tensor(scT[:, s, s * P:(s + 1) * P],
                                                psS[:, s * P:(s + 1) * P],
                                                m0_t[:, :], OP.mult)
                        if s < 3:
                            nc.scalar.copy(scT[:, s, (s + 1) * P:HALF],
                                           psS[:, (s + 1) * P:HALF])
                    else:
                        nc.scalar.mul(scT[:, s, :], psS[:, :], gsel_t[:, 1:2])

                for wave in range(2):
                    s_lo = wave * 4
                    tsl = slice(wave * HALF, (wave + 1) * HALF)
                    ps_list = []
                    for si in range(4):
                        psS = psp.tile([P, HALF], f32, tag=f"sc{si}",
                                       name=f"psS{s_lo + si}")
                        ps_list.append(psS)
                        scps[s_lo + si] = psS
                    for pc in range(NPC):
                        pcs = slice(pc * P, (pc + 1) * P)
                        psE = psp.tile([P, HALF], f32, tag="xspE", bufs=2)
                        psO = psp.tile([P, HALF], f32, tag="xspO", bufs=2)
                        nc.tensor.matmul(psE[:, :], wxe_a[:, pcs], xT0[:, tsl], start=True, stop=False)
                        nc.tensor.matmul(psE[:, :], wxe_b[64:P, pcs], xT1[64:P, tsl], start=False, stop=True)
                        nc.tensor.matmul(psO[:, :], wxo_a[:, pcs], xT0[:, tsl], start=True, stop=False)
                        nc.tensor.matmul(psO[:, :], wxo_b[64:P, pcs], xT1[64:P, tsl], start=False, stop=True)
                        if wave == 0:
                            Et = Eown[:, pc, :]
                            Ot = Oown[:, pc, :]
                        else:
                            Ew = work.tile([P, HALF], f16, tag="E1")
                            Ow = work.tile([P, HALF], f16, tag="O1")
                            Et = Ew[:, :]
                            Ot = Ow[:, :]
                        nc.scalar.activation(Et, psE[:, :], AF.Relu)
                        nc.scalar.activation(Ot, psO[:, :], AF.Relu)
                        t1 = work.tile([P, HALF], f16, tag="rt1")
                        t2 = work.tile([P, HALF], f16, tag="rt2")
                        nc.vector.tensor_mul(t1[:, :], Et, cos_t[:, pc, tsl])
                        nc.vector.tensor_mul(t2[:, :], Ot, sin_t[:, pc, tsl])
                        nc.vector.tensor_sub(qrE[:, pc, tsl], t1[:, :], t2[:, :])
                        t3 = work.tile([P, HALF], f16, tag="rt3")
                        t4 = work.tile([P, HALF], f16, tag="rt4")
                        nc.vector.tensor_mul(t3[:, :], Ot, cos_t[:, pc, tsl])
                        nc.vector.tensor_mul(t4[:, :], Et, sin_t[:, pc, tsl])
                        nc.vector.tensor_add(qrO[:, pc, tsl], t3[:, :], t4[:, :])
                        if pc >= LAG:
                            sc_mms(s_lo, ps_list, pc - LAG)
                    for pc in range(NPC - LAG, NPC):
                        sc_mms(s_lo, ps_list, pc)
                    for si in range(4):
                        sc_drain(s_lo + si)

                # ---- ykv + LN + transpose ----
                for tb in range(4):
                    tbs = slice(tb * P, (tb + 1) * P)
                    psY = psp.tile([P, D], f32, tag="xspE", bufs=2)
                    for s in range(TBLK):
                        nc.tensor.matmul(psY[:, :], scT[:, s, tbs], x16[:, s, :],
                                         start=(s == 0), stop=(s == TBLK - 1))
                    r, nmr = ln_vecs(psY[:, :])
                    ykvn = work.tile([P, D], f16, tag="ykvn")
                    nc.scalar.activation(ykvn[:, :], psY[:, :], AF.Identity,
                                         bias=nmr[:, :], scale=r[:, :])
                    nc.sync.dma_start(ykv_dram[tbs, :], ykvn[:, :])
                nc.sync.dma_start_transpose(ykvT0[:, :], ykv_dram[:, 0:P])
                nc.sync.dma_start_transpose(ykvT1[:, :], ykv_dram[:, 64:D])

                # ---- y_sparse, xy, mlp partial ----
                psM = []
                for _mi in range(4):
                    psM_t = psp.tile([P, D], f32, tag=f"sc{_mi}", name=f"psM{_mi}")
                    psM.append(psM_t)
                for side in range(2):
                    wa, wb = (wye_a, wye_b) if side == 0 else (wyo_a, wyo_b)
                    own = Eown if side == 0 else Oown
                    enc_t = ence_t if side == 0 else enco_t
                    for pc in range(NPC):
                        pcs = slice(pc * P, (pc + 1) * P)
                        psYS = psp.tile([P, HALF], f32, tag="xspO", bufs=2)
                        nc.tensor.matmul(psYS[:, :], wa[:, pcs], ykvT0[:, :], start=True, stop=False)
                        nc.tensor.matmul(psYS[:, :], wb[64:P, pcs], ykvT1[64:P, :], start=False, stop=True)
                        ys = work.tile([P, HALF], f16, tag="ys")
                        nc.scalar.activation(ys[:, :], psYS[:, :], AF.Relu)
                        xy = work.tile([P, HALF], f16, tag="xy")
                        nc.vector.tensor_mul(xy[:, :], own[:, pc, :], ys[:, :])
                        last = (side == 1 and pc == NPC - 1)
                        for tb in range(4):
                            nc.tensor.matmul(psM[tb][:, :], xy[:, tb * P:(tb + 1) * P],
                                             enc_t[:, pc, :],
                                             start=(side == 0 and pc == 0), stop=last)

                # ---- masked scatter to canonical bounce + AllReduce (fp16) ----
                for tb in range(4):
                    bA = work.tile([P, D], f16, tag="bA")
                    bB = work.tile([P, D], f16, tag="bB")
                    nc.scalar.mul(bA[:, :], psM[tb][:, :], gsel_t[:, 0:1])
                    nc.scalar.mul(bB[:, :], psM[tb][:, :], gsel_t[:, 1:2])
                    nc.sync.dma_start(bounce_in[tb * P:(tb + 1) * P, :], bA[:, :])
                    nc.sync.dma_start(bounce_in[HALF + tb * P:HALF + (tb + 1) * P, :], bB[:, :])
                nc.gpsimd.collective_compute(
                    "AllReduce", OP.add,
                    replica_groups=[list(range(NCORES))],
                    ins=[bounce_in.opt()],
                    outs=[bounce_out.opt()],
                )

                # ---- readback, ln(ymlp), residual, ln, new x16 ----
                for j in range(4):
                    Hj = work.tile([P, D], f16, tag="H0")
                    Hk = work.tile([P, D], f16, tag="H1")
                    nc.sync.dma_start(Hj[:, :], bounce_out[j * P:(j + 1) * P, :])
                    nc.sync.dma_start(Hk[:, :], bounce_out[HALF + j * P:HALF + (j + 1) * P, :])
                    r0, nm0 = ln_vecs(Hj[:, :])
                    r1, nm1 = ln_vecs(Hk[:, :])
                    scaled = []
                    for (rr, nn) in ((r0, nm0), (r1, nm1)):
                        for col in range(2):
                            rs = stp.tile([P, 1], f32, tag="rs")
                            ns = stp.tile([P, 1], f32, tag="ns")
                            nc.vector.tensor_mul(rs[:, :], rr[:, :], gsel_t[:, col:col + 1])
                            nc.vector.tensor_mul(ns[:, :], nn[:, :], gsel_t[:, col:col + 1])
                            scaled.append((rs, ns))
                    # scaled[0]=H0*s0, [1]=H0*s1, [2]=H1*s0, [3]=H1*s1
                    for slot, (ia, ib) in ((j, (0, 3)), (j + 4, (1, 2))):
                        t1a = work.tile([P, D], f32, tag="t1a")
                        t1b = work.tile([P, D], f32, tag="t1b")
                        ra, na = scaled[ia]
                        rb, nb = scaled[ib]
                        nc.scalar.activation(t1a[:, :], Hj[:, :], AF.Identity,
                                             bias=na[:, :], scale=ra[:, :])
                        nc.scalar.activation(t1b[:, :], Hk[:, :], AF.Identity,
                                             bias=nb[:, :], scale=rb[:, :])
                        xm = work.tile([P, D], f32, tag="xm")
                        nc.vector.tensor_add(xm[:, :], t1a[:, :], t1b[:, :])
                        nc.vector.tensor_add(xm[:, :], xm[:, :], x16[:, slot, :])
                        r2, nm2 = ln_vecs(xm[:, :])
                        nc.scalar.activation(x16[:, slot, :], xm[:, :], AF.Identity,
                                             bias=nm2[:, :], scale=r2[:, :])

            for rep in range(repeat):
                load_x_and_transpose(first=True)
                for li in range(N_LAYER):
                    layer()
                    load_x_and_transpose(first=False)

            # ---- lm head ----
            for tb in range(TBLK):
                tbs = slice(tb * P, (tb + 1) * P)
                psL = psp.tile([P, HALF], f32, tag="xspE", bufs=2)
                nc.tensor.matmul(psL[:, 0:VOCAB], xT0[:, tbs], lmh_a[:, :], start=True, stop=False)
                nc.tensor.matmul(psL[:, 0:VOCAB], xT1[64:P, tbs], lmh_b[64:P, :], start=False, stop=True)
                outL = work.tile([P, VOCAB], f32, tag="outL")
                nc.scalar.copy(outL[:, :], psL[:, 0:VOCAB])
                nc.sync.dma_start(logits_d[tbs, :], outL[:, :])

    nc.compile()
    _CACHE[key] = nc
    return nc


def make_inputs(idx, decoder_x, decoder_y, encoder, embed, pos_emb, lm_head):
    """Host-side prep: per-core input dicts (core c = head c//2, group c%2)."""
    idx = np.asarray(idx)
    decoder_x = np.asarray(decoder_x, dtype=np.float32)
    decoder_y = np.asarray(decoder_y, dtype=np.float32)
    encoder = np.asarray(encoder, dtype=np.float32).reshape(NH, N, D)
    embed = np.asarray(embed, dtype=np.float32)
    pos_emb = np.asarray(pos_emb, dtype=np.float32)
    lm_head = np.asarray(lm_head, dtype=np.float32)

    x0 = _ln_np(embed[idx[0]] + pos_emb[:T]).astype(np.float16)

    freqs = _get_freqs(N)
    fpair = freqs[0::2]
    tt = np.arange(T, dtype=np.float32)
    m0 = np.triu(np.ones((P, P), np.float32), k=1).astype(np.float16)
    lmh = lm_head.astype(np.float16)

    in_maps = []
    for c in range(NCORES):
        h, g = c // 2, c % 2
        tperm = tt if g == 0 else np.concatenate([tt[HALF:], tt[:HALF]])
        ph = ((fpair[:, None] * tperm[None, :]).astype(np.float32) % 1.0) \
            * np.float32(2.0 * math.pi)
        gsel = np.zeros((P, 2), np.float32)
        gsel[:, 0] = 1.0 if g == 0 else 0.0
        gsel[:, 1] = 1.0 - gsel[:, 0]
        x0c = x0 if g == 0 else np.concatenate([x0[HALF:], x0[:HALF]])
        in_maps.append({
            "x0": np.ascontiguousarray(x0c),
            "wxe": np.ascontiguousarray(decoder_x[h][:, 0::2]).astype(np.float16),
            "wxo": np.ascontiguousarray(decoder_x[h][:, 1::2]).astype(np.float16),
            "wye": np.ascontiguousarray(decoder_y[h][:, 0::2]).astype(np.float16),
            "wyo": np.ascontiguousarray(decoder_y[h][:, 1::2]).astype(np.float16),
            "ence": np.ascontiguousarray(encoder[h][0::2]).astype(np.float16),
            "enco": np.ascontiguousarray(encoder[h][1::2]).astype(np.float16),
            "cosT": np.cos(ph.astype(np.float64)).astype(np.float16),
            "sinT": np.sin(ph.astype(np.float64)).astype(np.float16),
            "m0": m0,
            "gsel": gsel,
            "lmh": lmh,
        })
    return in_maps


def kernel(idx, decoder_x, decoder_y, encoder, embed, pos_emb, lm_head):
    from concourse.bass_utils import run_bass_kernel_spmd

    nc = build_program()
    in_maps = make_inputs(idx, decoder_x, decoder_y, encoder, embed, pos_emb,
                          lm_head)
    res = run_bass_kernel_spmd(nc, in_maps, list(range(NCORES)))
    logits = res.results[0]["logits"]  # core 0 uses the identity permutation
    return logits.reshape(1, T, VOCAB).astype(np.float32)
